# revision 1
# baseline (speedup 1.0000x reference)
"""Correct-and-Smooth label propagation on 8 Trainium2 NeuronCores.

Strategy: destination-node row sharding. Nodes are relabeled (degree-balanced
round-robin across cores, degree-sorted within a core, lane-major within each
128-row tile) so each core owns a contiguous block of the replicated [N,C]
state table. Per propagation step each core:
  1. gathers source rows for its incoming edges with one indirect DMA per
     uniform-K group of destination tiles (edge slot tables are padded so the
     segment-sum becomes a fixed-stride reduction),
  2. multiplies by edge norms and segment-reduces on the vector engine,
  3. applies the alpha-mix + clip/softmax post-step,
  4. AllGathers the updated shards into the next replicated table.
"""
import numpy as np

N, E, C, M = 100000, 1600000, 40, 8
NPC, TPC = 12544, 98          # padded nodes per core, 128-row tiles per core
ALPHA_C, NUM_C = 0.9, 10
ALPHA_S, NUM_S = 0.8, 10
SLOT_CAP = 160                # max edge slots per partition per gather group
GROUP_PEN = 12
CS = 44                       # padded slot stride (elems); slot pad breaks DMA
                              # coalescing so each slot gets its own descriptor

_cache = {}


def _group_tiles(Ktile, cap=SLOT_CAP, pen=GROUP_PEN):
    T = len(Ktile)
    INF = 1 << 60
    best = [INF] * (T + 1)
    prev = [-1] * (T + 1)
    best[0] = 0
    for i in range(1, T + 1):
        mk = 0
        for j in range(i - 1, -1, -1):
            mk = max(mk, Ktile[j])
            G = i - j
            if G * mk > cap:
                break
            cst = best[j] + G * mk + pen
            if cst < best[i]:
                best[i] = cst
                prev[i] = j
    out = []
    i = T
    while i > 0:
        j = prev[i]
        out.append((j, i - j, int(max(Ktile[j:i]))))
        i = j
    return out[::-1]


def _preprocess(y_true, y_soft, spread_mask, edge_index, edge_weight):
    y_true = np.asarray(y_true)
    y_soft = np.asarray(y_soft, dtype=np.float32)
    spread_mask = np.asarray(spread_mask).astype(bool)
    row = np.asarray(edge_index[0], dtype=np.int64)
    col = np.asarray(edge_index[1], dtype=np.int64)
    w = np.asarray(edge_weight, dtype=np.float32)

    deg = np.bincount(row, weights=w.astype(np.float64), minlength=N).astype(np.float32)
    dis = np.where(deg > 0, 1.0 / np.sqrt(deg, where=deg > 0), 0.0).astype(np.float32)
    norm = (dis[row] * w * dis[col]).astype(np.float32)

    indeg = np.bincount(row, minlength=N)
    order = np.argsort(indeg, kind="stable")
    ranks = np.arange(N)
    core_of = np.empty(N, np.int64)
    pos_of = np.empty(N, np.int64)
    core_of[order] = ranks % M
    pos_of[order] = ranks // M
    lane_of = pos_of % 128
    t_of = pos_of // 128
    new_id = core_of * NPC + lane_of * TPC + t_of   # lane-major within core

    sc = new_id[col].astype(np.int32)
    destkey = (core_of[row] * TPC + t_of[row]) * 128 + lane_of[row]
    eo = np.argsort(destkey, kind="stable")
    dk_s = destkey[eo]
    sc_s = sc[eo]
    nm_s = norm[eo]
    cnt = np.bincount(dk_s, minlength=M * TPC * 128)
    starts = np.zeros(M * TPC * 128 + 1, np.int64)
    np.cumsum(cnt, out=starts[1:])
    slot = np.arange(E) - starts[dk_s]

    Ktile = cnt.reshape(M, TPC, 128).max(axis=2).max(axis=0)
    groups = _group_tiles(Ktile)
    tile_off = np.zeros(TPC, np.int64)
    off = 0
    for (t0, G, Kg) in groups:
        for t in range(t0, t0 + G):
            tile_off[t] = off + (t - t0) * Kg
        off += G * Kg
    TOT = off

    idx_all = np.zeros((M, 128, TOT), np.int32)
    nrm_all = np.zeros((M, 128, TOT), np.float32)
    e_core = dk_s // (TPC * 128)
    e_t = (dk_s // 128) % TPC
    e_lane = dk_s % 128
    epos = tile_off[e_t] + slot
    idx_all[e_core, e_lane, epos] = sc_s
    nrm_all[e_core, e_lane, epos] = nm_s

    y_oh = np.zeros((N, C), np.float32)
    y_oh[np.arange(N), y_true] = 1.0
    maskf = spread_mask[:, None]
    err = np.where(maskf, y_oh - y_soft, 0.0).astype(np.float32)
    sigma = float(np.abs(err).sum(dtype=np.float64) / spread_mask.sum())

    x0 = np.zeros((M * NPC, C), np.float32)
    x0[new_id] = err
    A = np.zeros((M * NPC, C), np.float32)
    A[new_id] = np.where(maskf, y_oh, y_soft)
    B = np.zeros((M * NPC,), np.float32)
    B[new_id] = (~spread_mask).astype(np.float32)

    return dict(idx_all=idx_all, nrm_all=nrm_all, groups=groups, TOT=TOT,
                x0=x0, A=A, B=B, sigma=sigma, new_id=new_id)


def _build_program(groups, TOT, sigma):
    import concourse.bass as bass
    import concourse.bacc as bacc
    import concourse.mybir as mybir
    import concourse.tile as tile

    f32 = mybir.dt.float32
    i32 = mybir.dt.int32
    Alu = mybir.AluOpType
    Act = mybir.ActivationFunctionType
    X = mybir.AxisListType.X

    nc = bacc.Bacc("TRN2", target_bir_lowering=False, debug=False,
                   enable_asserts=True, num_devices=M)
    x0_d = nc.dram_tensor("x0", [M * NPC, C], f32, kind="ExternalInput")
    xo0_d = nc.dram_tensor("xo0", [128, TPC * C], f32, kind="ExternalInput")
    idx_d = nc.dram_tensor("idx", [128, TOT], i32, kind="ExternalInput")
    nrm_d = nc.dram_tensor("nrm", [128, TOT], f32, kind="ExternalInput")
    a_d = nc.dram_tensor("amat", [128, TPC * C], f32, kind="ExternalInput")
    b_d = nc.dram_tensor("bvec", [128, TPC], f32, kind="ExternalInput")
    out_d = nc.dram_tensor("out", [128, TPC * C], f32, kind="ExternalOutput")

    NSTEPS = NUM_C + NUM_S
    with tile.TileContext(nc) as tc:
        with (
            tc.tile_pool(name="stat", bufs=1) as stat,
            tc.tile_pool(name="gpool", bufs=2) as gpool,
            tc.tile_pool(name="spool", bufs=3) as spool,
            tc.tile_pool(name="apool", bufs=2) as apool,
            tc.tile_pool(name="smp", bufs=2) as smp,
            tc.tile_pool(name="dpool", bufs=2, space="DRAM") as dpool,
        ):
            idx_t = stat.tile([128, TOT], i32)
            nc.sync.dma_start(out=idx_t[:], in_=idx_d[:])
            nrm_t = stat.tile([128, TOT], f32)
            nc.sync.dma_start(out=nrm_t[:], in_=nrm_d[:])
            a_t = stat.tile([128, TPC * C], f32)
            nc.sync.dma_start(out=a_t[:], in_=a_d[:])
            b_t = stat.tile([128, TPC], f32)
            nc.sync.dma_start(out=b_t[:], in_=b_d[:])
            xcur = spool.tile([128, TPC * C], f32, tag="xst")
            nc.sync.dma_start(out=xcur[:], in_=xo0_d[:])
            src = x0_d.ap()

            for step in range(NSTEPS):
                phase1 = step < NUM_C
                alpha = ALPHA_C if phase1 else ALPHA_S

                agg_t = apool.tile([128, TPC * C], f32, tag="agg")
                off = 0
                for (t0, G, Kg) in groups:
                    S = G * Kg
                    g_t = gpool.tile([128, S * C], f32, tag="gath")
                    for r in range(S):
                        nc.gpsimd.indirect_dma_start(
                            out=g_t[:, r * C:(r + 1) * C], out_offset=None, in_=src,
                            in_offset=bass.IndirectOffsetOnAxis(
                                ap=idx_t[:, off + r:off + r + 1], axis=0))
                    gv = g_t[:].rearrange("p (s c) -> p s c", c=C)
                    nv = nrm_t[:, off:off + S].unsqueeze(-1).broadcast_to([128, S, C])
                    nc.vector.tensor_tensor(out=gv, in0=gv, in1=nv, op=Alu.mult)
                    gr = g_t[:].rearrange("p (g k c) -> p g c k", k=Kg, c=C)
                    nc.vector.tensor_reduce(
                        out=agg_t[:, t0 * C:(t0 + G) * C], in_=gr, axis=X, op=Alu.add)
                    off += S

                xn = spool.tile([128, TPC * C], f32, tag="xst")
                nc.scalar.activation(xn[:], xcur[:], Act.Copy, scale=float(1 - alpha))
                nc.vector.tensor_scalar_mul(out=agg_t[:], in0=agg_t[:],
                                            scalar1=float(alpha))
                nc.vector.tensor_add(out=xn[:], in0=xn[:], in1=agg_t[:])

                if phase1:
                    nc.vector.tensor_scalar_max(out=xn[:], in0=xn[:], scalar1=-1.0)
                    nc.vector.tensor_scalar_min(out=xn[:], in0=xn[:], scalar1=1.0)
                else:
                    xv = xn[:].rearrange("p (t c) -> p t c", c=C)
                    rm = smp.tile([128, TPC], f32, tag="rm")
                    nc.vector.tensor_reduce(out=rm[:], in_=xv, axis=X, op=Alu.max)
                    rmb = rm[:].unsqueeze(-1).broadcast_to([128, TPC, C])
                    nc.vector.tensor_tensor(out=xv, in0=xv, in1=rmb, op=Alu.subtract)
                    nc.scalar.activation(xn[:], xn[:], Act.Exp)
                    ss = smp.tile([128, TPC], f32, tag="ss")
                    nc.vector.tensor_reduce(out=ss[:], in_=xv, axis=X, op=Alu.add)
                    nc.vector.reciprocal(out=ss[:], in_=ss[:])
                    ssb = ss[:].unsqueeze(-1).broadcast_to([128, TPC, C])
                    nc.vector.tensor_tensor(out=xv, in0=xv, in1=ssb, op=Alu.mult)

                if step == NUM_C - 1:
                    # correct/smooth transition: xn holds `smoothed`
                    xv = xn[:].rearrange("p (t c) -> p t c", c=C)
                    den = smp.tile([128, TPC], f32, tag="den")
                    nc.vector.tensor_reduce(out=den[:], in_=xv, axis=X, op=Alu.add,
                                            apply_absolute_value=True)
                    raw = smp.tile([128, TPC], f32, tag="raw")
                    nc.vector.reciprocal(out=raw[:], in_=den[:])
                    nc.vector.tensor_scalar_mul(out=raw[:], in0=raw[:],
                                                scalar1=float(sigma))
                    # scale = where((den>0) & (raw<=1000), raw, 1), branch-free:
                    # clamp raw first so inf (den==0) never meets a 0 multiply
                    nc.vector.tensor_scalar_min(out=raw[:], in0=raw[:],
                                                scalar1=1001.0)
                    m1 = smp.tile([128, TPC], f32, tag="m1")
                    nc.vector.tensor_scalar(out=m1[:], in0=den[:], scalar1=0.0,
                                            scalar2=None, op0=Alu.is_gt)
                    m2 = smp.tile([128, TPC], f32, tag="m2")
                    nc.vector.tensor_scalar(out=m2[:], in0=raw[:], scalar1=1000.0,
                                            scalar2=None, op0=Alu.is_le)
                    nc.vector.tensor_tensor(out=m1[:], in0=m1[:], in1=m2[:],
                                            op=Alu.mult)
                    scl = smp.tile([128, TPC], f32, tag="scl")
                    nc.vector.tensor_scalar_add(out=raw[:], in0=raw[:], scalar1=-1.0)
                    nc.vector.tensor_tensor(out=scl[:], in0=raw[:], in1=m1[:],
                                            op=Alu.mult)
                    nc.vector.tensor_scalar_add(out=scl[:], in0=scl[:], scalar1=1.0)
                    nc.vector.tensor_tensor(out=scl[:], in0=scl[:], in1=b_t[:],
                                            op=Alu.mult)
                    ys = spool.tile([128, TPC * C], f32, tag="xst")
                    yv = ys[:].rearrange("p (t c) -> p t c", c=C)
                    sclb = scl[:].unsqueeze(-1).broadcast_to([128, TPC, C])
                    nc.vector.tensor_tensor(out=yv, in0=xv, in1=sclb, op=Alu.mult)
                    nc.vector.tensor_add(out=ys[:], in0=ys[:], in1=a_t[:])
                    xn = ys

                if step < NSTEPS - 1:
                    agin = dpool.tile([128, TPC * C], f32, tag="agin")
                    nc.sync.dma_start(out=agin[:], in_=xn[:])
                    xrep = dpool.tile([M * NPC, C], f32, tag="xrep",
                                      addr_space="Shared")
                    nc.gpsimd.collective_compute(
                        "AllGather", Alu.bypass,
                        replica_groups=[list(range(M))],
                        ins=[agin.opt()], outs=[xrep.opt()])
                    src = xrep
                xcur = xn

            nc.sync.dma_start(out=out_d[:], in_=xcur[:])
    nc.compile()
    return nc


def _make_in_maps(pp):
    in_maps = []
    for k in range(M):
        xo = pp["x0"][k * NPC:(k + 1) * NPC].reshape(128, TPC * C)
        am = pp["A"][k * NPC:(k + 1) * NPC].reshape(128, TPC * C)
        bv = pp["B"][k * NPC:(k + 1) * NPC].reshape(128, TPC)
        in_maps.append({
            "x0": pp["x0"],
            "xo0": np.ascontiguousarray(xo),
            "idx": pp["idx_all"][k],
            "nrm": pp["nrm_all"][k],
            "amat": np.ascontiguousarray(am),
            "bvec": np.ascontiguousarray(bv),
        })
    return in_maps


TRACE = False
LAST_EXEC_NS = None
LAST_RESULTS = None


def kernel(y_true, y_soft, spread_mask, edge_index, edge_weight):
    global LAST_EXEC_NS, LAST_RESULTS
    from concourse import bass_utils

    pp = _preprocess(y_true, y_soft, spread_mask, edge_index, edge_weight)
    key = (tuple(pp["groups"]), pp["TOT"], round(pp["sigma"], 9))
    if key not in _cache:
        _cache[key] = _build_program(pp["groups"], pp["TOT"], pp["sigma"])
    nc = _cache[key]

    res = bass_utils.run_bass_kernel_spmd(nc, _make_in_maps(pp), list(range(M)),
                                          trace=TRACE)
    LAST_EXEC_NS = res.exec_time_ns
    LAST_RESULTS = res
    full = np.concatenate(
        [res.results[k]["out"].reshape(NPC, C) for k in range(M)], axis=0)
    return full[pp["new_id"]].astype(np.float32)

